# revision 27
# baseline (speedup 1.0000x reference)
"""CKA (RBF-kernel HSIC) on 8 Trainium2 NeuronCores.

Row-shards the n=4096 samples across 8 cores. Each core computes its
[512, 4096] slab of both RBF Gram matrices tile-by-tile on-chip (never
materialized in DRAM) and emits only:
  - per-row sums of Kx and Ky               (rx, ry)
  - partial sums  S_xx = sum Kx*Kx, S_yy, S_xy  over its slab
The host combines partials in float64 via
  HSIC(Ka,Kb) = S_ab - (2/n) ra.rb + (sum Ka)(sum Kb)/n^2
which is algebraically identical to sum(center(Ka)*center(Kb)).

Device details:
  - Gram matmuls run in fp8-e4m3 DoubleRow mode (2 contraction rows per
    partition -> 2x PE throughput, 0.5 cycles per output column).
  - Four extra fp8 contraction rows (hi, hi, mid, lo) carry -(|x_j|^2)/2
    as a cascaded-residual decomposition, folded in as one more DR
    matmul per 512-column half, so exp(G/s^2 + bias_i) with
    bias_i = -|x_i|^2/(2 s^2) yields the full RBF kernel in a single
    ScalarE activation whose accum_out produces row sums for free.
  - PSUM chunks are [128, 2048] (4 banks, double buffered = all 8).
  - Work is phased Y-c0, X-c0, Y-c1, X-c1 (chunk-outer) with DMA in the
    same order, so the three Hadamard-product sums start as early as
    possible and overlap the Gram/exp pipeline.  They run on VectorE
    (scalar_tensor_tensor with fp32 accum), except the last few sum
    Kx*Kx chunks, which run as ScalarE Square activations to fill ACT's
    tail while VectorE drains (GpSimd rejects TensorScalarPtr on HW).
"""

import numpy as np
import ml_dtypes

BF16 = ml_dtypes.bfloat16
FP8 = ml_dtypes.float8_e4m3

N = 4096          # samples
D = 768           # features
NCORES = 8
ROWS = N // NCORES        # 512 rows per core
MT = ROWS // 128          # 4 m-tiles per core
KC2 = D // 256            # 3 DoubleRow contraction chunks (256 rows each)
ACHUNK = 2048             # ACT/psum chunk width (4 PSUM banks)
NCH = N // ACHUNK         # 2 chunks per m-tile row
MMN = 512                 # matmul moving free dim (one PSUM bank)
NHALF = ACHUNK // MMN     # 4 matmul halves per chunk
NACC = MT * NCH           # row-sum accumulator columns (8)
SACC = MT * 4             # product accumulator columns (4 parts per slot)

_cache = {}
LAST_RESULTS = None   # BassKernelResults of the most recent run (for test harness)


def _build(inv_sigma_sq: float, reps: int = 1, stages: str = "all"):
    import concourse.bacc as bacc
    import concourse.mybir as mybir
    import concourse.tile as tile

    fp32 = mybir.dt.float32
    bf16 = mybir.dt.bfloat16
    fp8 = mybir.dt.float8e4
    DR = mybir.MatmulPerfMode.DoubleRow
    Exp = mybir.ActivationFunctionType.Exp
    mult = mybir.AluOpType.mult

    nc = bacc.Bacc(None)

    xt = nc.dram_tensor("xt", [KC2, 128, 2, N], fp8, kind="ExternalInput")
    yt = nc.dram_tensor("yt", [KC2, 128, 2, N], fp8, kind="ExternalInput")
    # stationary slabs, partition-major so each loads in ONE dma
    xbt = nc.dram_tensor("xbt", [128, KC2, 2, ROWS], fp8, kind="ExternalInput")
    ybt = nc.dram_tensor("ybt", [128, KC2, 2, ROWS], fp8, kind="ExternalInput")
    # aug rows for x (cols 0:N), y (N:2N), plus all-ones stationary (2N:2N+128)
    augc = nc.dram_tensor("augc", [2, 2, 2 * N + 128], fp8,
                          kind="ExternalInput")
    biasc = nc.dram_tensor("biasc", [128, 2 * MT], fp32, kind="ExternalInput")

    rx_o = nc.dram_tensor("rx", [128, NACC], fp32, kind="ExternalOutput")
    ry_o = nc.dram_tensor("ry", [128, NACC], fp32, kind="ExternalOutput")
    sxx_o = nc.dram_tensor("sxx", [128, SACC], fp32, kind="ExternalOutput")
    syy_o = nc.dram_tensor("syy", [128, SACC], fp32, kind="ExternalOutput")
    sxy_o = nc.dram_tensor("sxy", [128, SACC], fp32, kind="ExternalOutput")

    with tile.TileContext(nc) as tc:
        with (
            tc.tile_pool(name="res", bufs=1) as res,
            tc.tile_pool(name="kmat", bufs=1) as kpool,
            tc.tile_pool(name="scr", bufs=1) as spool,
            tc.tile_pool(name="psum", bufs=2, space="PSUM") as pp,
        ):
            # ---- persistent tiles ----
            t_sb = {}      # moving operands [128, 2, N] per chunk
            bt_sb = {}     # stationary slabs [128, KC2, 2, ROWS]
            for mat in ("x", "y"):
                for k in range(KC2):
                    t_sb[mat, k] = res.tile([128, 2, N], fp8,
                                            tag=f"{mat}t{k}", name=f"{mat}t{k}")
                bt_sb[mat] = res.tile([128, KC2, 2, ROWS], fp8,
                                      tag=f"{mat}bt", name=f"{mat}bt")
            augc_sb = res.tile([2, 2, 2 * N + 128], fp8, tag="augc",
                               name="augc_sb")
            biasc_sb = res.tile([128, 2 * MT], fp32, tag="biasc",
                                name="biasc_sb")
            aug_sb = {"x": augc_sb[:, :, 0:N], "y": augc_sb[:, :, N:2 * N]}
            augst_ap = augc_sb[:, :, 2 * N:2 * N + 128]
            bias_sb = {"x": biasc_sb[:, 0:MT], "y": biasc_sb[:, MT:2 * MT]}

            racc = {
                "x": res.tile([128, NACC], fp32, tag="rxacc", name="rxacc"),
                "y": res.tile([128, NACC], fp32, tag="ryacc", name="ryacc"),
            }
            sxx_acc = res.tile([128, SACC], fp32, tag="sxxacc", name="sxxacc")
            syy_acc = res.tile([128, SACC], fp32, tag="syyacc", name="syyacc")
            sxy_acc = res.tile([128, SACC], fp32, tag="sxyacc", name="sxyacc")

            def load():
                # Constants + stationaries first, then moving columns in
                # compute-phase order: Y-lo, X-lo, Y-hi, X-hi.
                nc.sync.dma_start(biasc_sb[:], biasc[:])
                nc.sync.dma_start(augc_sb[:], augc[:])
                nc.sync.dma_start(bt_sb["y"][:], ybt[:])
                sl0 = slice(0, ACHUNK)
                for k in range(KC2):
                    nc.sync.dma_start(t_sb["y", k][:, :, sl0], yt[k][:, :, sl0])
                nc.sync.dma_start(bt_sb["x"][:], xbt[:])
                for k in range(KC2):
                    nc.sync.dma_start(t_sb["x", k][:, :, sl0], xt[k][:, :, sl0])
                sl1 = slice(ACHUNK, N)
                for mat, tdram in (("y", yt), ("x", xt)):
                    for k in range(KC2):
                        nc.sync.dma_start(t_sb[mat, k][:, :, sl1],
                                          tdram[k][:, :, sl1])

            def gram_exp_ap(mat, m, out_ap, nch):
                """RBF kernel chunk: rows [m*128,(m+1)*128) x cols chunk nch,
                written to out_ap ([128, ACHUNK])."""
                g = pp.tile([128, ACHUNK], fp32, tag="g", name="g")
                for k in range(KC2):
                    stat = bt_sb[mat][:, k, :, m * 128:(m + 1) * 128]
                    for h in range(NHALF):
                        base = nch * ACHUNK + h * MMN
                        nc.tensor.matmul(
                            g[:, h * MMN:(h + 1) * MMN],
                            stat,
                            t_sb[mat, k][:, :, base:base + MMN],
                            start=(k == 0),
                            stop=False,
                            perf_mode=DR,
                        )
                for h in range(NHALF):
                    base = nch * ACHUNK + h * MMN
                    nc.tensor.matmul(
                        g[:, h * MMN:(h + 1) * MMN],
                        augst_ap,
                        aug_sb[mat][:, :, base:base + MMN],
                        start=False,
                        stop=True,
                        perf_mode=DR,
                    )
                col = m * NCH + nch
                nc.scalar.activation(
                    out_ap,
                    g[:],
                    Exp,
                    bias=bias_sb[mat][:, m:m + 1],
                    scale=inv_sigma_sq,
                    accum_out=racc[mat][:, col:col + 1],
                )

            def body():
                if stages in ("all", "dma"):
                    load()
                if stages == "dma":
                    return

                ky = {
                    m: kpool.tile([128, N], bf16, tag=f"ky{m}", name=f"ky{m}")
                    for m in range(MT)
                }
                kx = {
                    m: kpool.tile([128, N], bf16, tag=f"kx{m}", name=f"kx{m}")
                    for m in range(MT)
                }

                def part_ranges(m, nch):
                    """Symmetric product sub-ranges of chunk `nch` for slot m.

                    With each core's moving columns pre-rotated by -c*ROWS,
                    slot m's tile column j' maps to global column tile
                    (4c + m + j'//128 - m) ... i.e. offset d = j'//128 - m.
                    Products cover d = 0..16: d=0 (own diagonal block) and
                    d=16 (computed by both mirror owners) at weight 1,
                    d=1..15 (mirror owner skips) at weight 2.
                    Returns (start, width, acc_col) triples within `nch`.
                    """
                    out = []
                    if nch == 0:
                        out.append((m * 128, 128, m * 4 + 0))          # d=0
                        out.append(((m + 1) * 128,
                                    ACHUNK - (m + 1) * 128, m * 4 + 1))
                    else:
                        w2b = (m + 16) * 128 - ACHUNK                  # d<16 tail
                        if w2b > 0:
                            out.append((ACHUNK, w2b, m * 4 + 2))
                        out.append(((m + 16) * 128, 128, m * 4 + 3))   # d=16
                    return out

                def prods(m, nch, pairs):
                    """pairs: list of (in0_tile, in1_tile, acc_tile)."""
                    for st, w, col in part_ranges(m, nch):
                        for in0, in1, acc in pairs:
                            dscr = spool.tile([128, ACHUNK], bf16,
                                              tag="dscr", name="dscr",
                                              bufs=3)
                            nc.vector.scalar_tensor_tensor(
                                out=dscr[:, 0:w],
                                in0=in0[:, st:st + w], scalar=1.0,
                                in1=in1[:, st:st + w], op0=mult, op1=mult,
                                accum_out=acc[:, col:col + 1],
                            )

                def do_y(m, nch):
                    sl = slice(nch * ACHUNK, (nch + 1) * ACHUNK)
                    gram_exp_ap("y", m, ky[m][:, sl], nch)
                    prods(m, nch, [(ky[m], ky[m], syy_acc)])

                def do_x(m, nch):
                    sl = slice(nch * ACHUNK, (nch + 1) * ACHUNK)
                    gram_exp_ap("x", m, kx[m][:, sl], nch)
                    prods(m, nch, [(kx[m], kx[m], sxx_acc),
                                   (kx[m], ky[m], sxy_acc)])

                # Chunk phase 0: Y leads by two m-tiles (its data lands
                # first); X m-tiles interleave as xt-lo arrives.
                order0 = [("y", 0), ("y", 1), ("x", 0), ("y", 2), ("x", 1),
                          ("y", 3), ("x", 2), ("x", 3)]
                order1 = [("y", 0), ("x", 0), ("y", 1), ("x", 1), ("y", 2),
                          ("x", 2), ("y", 3), ("x", 3)]
                for nch, order in ((0, order0), (1, order1)):
                    for mat, m in order:
                        if mat == "y":
                            do_y(m, nch)
                        else:
                            do_x(m, nch)

            if stages == "compute":
                load()
            if reps == 1:
                body()
            elif reps < 0:          # unrolled: -reps copies, no hw loop
                for _ in range(-reps):
                    body()
            else:
                with tc.For_i(0, reps, 1):
                    body()

            if stages != "dma":
                nc.sync.dma_start(rx_o[:], racc["x"][:])
                nc.sync.dma_start(ry_o[:], racc["y"][:])
                nc.sync.dma_start(sxx_o[:], sxx_acc[:])
                nc.sync.dma_start(syy_o[:], syy_acc[:])
                nc.sync.dma_start(sxy_o[:], sxy_acc[:])

    if not nc.is_finalized():
        nc.finalize()
    return nc


def _prep_matrix(A, inv_sigma_sq):
    """Host-side: fp8 cast, transpose+DoubleRow layout, fp8 aug rows, bias."""
    A8 = A.astype(FP8)
    Af = A8.astype(np.float64)
    d = (Af ** 2).sum(axis=1)                             # [N] row norms^2
    AT = np.ascontiguousarray(A8.T)                       # [D, N] fp8

    # -(d_j)/2 as 4 cascaded fp8 rows (a0 + a0 + a2 + a3), |a0| <= 240
    a0 = (-0.25 * d).astype(FP8)
    rem = -0.5 * d - 2.0 * a0.astype(np.float64)
    a2 = rem.astype(FP8)
    rem2 = rem - a2.astype(np.float64)
    a3 = rem2.astype(FP8)
    aug = np.empty((2, 2, N), dtype=FP8)
    aug[0, 0] = a0
    aug[0, 1] = a0
    aug[1, 0] = a2
    aug[1, 1] = a3

    # bias uses the SAME fp8-cascade value as the aug rows so the computed
    # exponent (G + ahat_j)/s^2 + ahat_i/s^2 is symmetric in (i, j) — the
    # symmetric product reconstruction double-counts one triangle, so any
    # aug-vs-bias mismatch would break K's symmetry at the ~1e-3 level.
    ahat = (2.0 * a0.astype(np.float64) + a2.astype(np.float64)
            + a3.astype(np.float64))                      # ~= -d/2
    bias = (inv_sigma_sq * ahat).astype(np.float32)       # [N]
    return AT, aug, bias


def _dr_layout(AT_slice):
    """[768, W] fp8 -> [KC2, 128, 2, W] DoubleRow layout (row = i*128+p)."""
    W = AT_slice.shape[1]
    return np.ascontiguousarray(
        AT_slice.reshape(KC2, 2, 128, W).transpose(0, 2, 1, 3))


def _make_in_maps(X, Y, inv_sigma_sq):
    XT, xaug, xbias = _prep_matrix(X, inv_sigma_sq)
    YT, yaug, ybias = _prep_matrix(Y, inv_sigma_sq)
    xt_r = _dr_layout(XT)
    yt_r = _dr_layout(YT)

    augc = np.ones((2, 2, 2 * N + 128), dtype=FP8)
    augc[:, :, 0:N] = xaug
    augc[:, :, N:2 * N] = yaug

    in_maps = []
    for c in range(NCORES):
        sl = slice(c * ROWS, (c + 1) * ROWS)
        biasc = np.empty((128, 2 * MT), dtype=np.float32)
        biasc[:, 0:MT] = xbias[sl].reshape(MT, 128).T
        biasc[:, MT:2 * MT] = ybias[sl].reshape(MT, 128).T
        # Rotate this core's moving columns left by c*ROWS so the
        # symmetric product ranges [m*128, (m+17)*128) are the same AP
        # on every core (kernel column j' = global (j' + c*ROWS) % N).
        sh = -c * ROWS
        augc_c = augc.copy()
        augc_c[:, :, 0:N] = np.roll(xaug, sh, axis=-1)
        augc_c[:, :, N:2 * N] = np.roll(yaug, sh, axis=-1)
        in_maps.append({
            "xt": np.ascontiguousarray(np.roll(xt_r, sh, axis=-1)),
            "yt": np.ascontiguousarray(np.roll(yt_r, sh, axis=-1)),
            "xbt": np.ascontiguousarray(
                _dr_layout(XT[:, sl]).transpose(1, 0, 2, 3)),
            "ybt": np.ascontiguousarray(
                _dr_layout(YT[:, sl]).transpose(1, 0, 2, 3)),
            "augc": augc_c,
            "biasc": np.ascontiguousarray(biasc),
        })
    return in_maps


def _combine(out):
    rx = np.empty(N, dtype=np.float64)
    ry = np.empty(N, dtype=np.float64)
    s_xx = s_yy = s_xy = 0.0
    for c in range(NCORES):
        r = out[c]
        rxc = r["rx"].astype(np.float64).reshape(128, MT, NCH).sum(axis=2)
        ryc = r["ry"].astype(np.float64).reshape(128, MT, NCH).sum(axis=2)
        rx[c * ROWS:(c + 1) * ROWS] = rxc.T.reshape(ROWS)
        ry[c * ROWS:(c + 1) * ROWS] = ryc.T.reshape(ROWS)
        # product accs: 4 parts per slot with symmetry weights
        # (own-diagonal block 1, offsets 1..15 doubled, offset 16 once).
        # Slot 0's part 2 (w2b) is empty and never written on device —
        # mask out whatever stale SBUF contents it holds.
        w = np.tile(np.array([1.0, 2.0, 2.0, 1.0]), MT)
        w[2] = 0.0
        s_xx += (r["sxx"].astype(np.float64).sum(axis=0) * w).sum()
        s_yy += (r["syy"].astype(np.float64).sum(axis=0) * w).sum()
        s_xy += (r["sxy"].astype(np.float64).sum(axis=0) * w).sum()

    tx = rx.sum()
    ty = ry.sum()
    n = float(N)
    hsic_xy = s_xy - 2.0 / n * np.dot(rx, ry) + tx * ty / (n * n)
    hsic_xx = s_xx - 2.0 / n * np.dot(rx, rx) + tx * tx / (n * n)
    hsic_yy = s_yy - 2.0 / n * np.dot(ry, ry) + ty * ty / (n * n)
    return np.float32(hsic_xy / np.sqrt(hsic_xx * hsic_yy))


def kernel(X, Y, sigma, _reps=1):
    import os
    from concourse.bass_utils import run_bass_kernel_spmd

    X = np.asarray(X, dtype=np.float32)
    Y = np.asarray(Y, dtype=np.float32)
    sig = float(np.asarray(sigma))
    inv_sigma_sq = 1.0 / (sig * sig)

    stages = os.environ.get("KERNEL_STAGES", "all")
    key = (inv_sigma_sq, _reps, stages)
    if key not in _cache:
        _cache[key] = _build(inv_sigma_sq, reps=_reps, stages=stages)
    nc = _cache[key]

    in_maps = _make_in_maps(X, Y, inv_sigma_sq)
    res = run_bass_kernel_spmd(nc, in_maps, list(range(NCORES)))
    global LAST_RESULTS
    LAST_RESULTS = res
    return _combine(res.results)


# revision 29
# speedup vs baseline: 1.2803x; 1.2803x over previous
"""CKA (RBF-kernel HSIC) on 8 Trainium2 NeuronCores.

Row-shards the n=4096 samples across 8 cores. Each core computes its
[512, 4096] slab of both RBF Gram matrices tile-by-tile on-chip (never
materialized in DRAM) and emits only:
  - per-row sums of Kx and Ky               (rx, ry)
  - partial sums  S_xx = sum Kx*Kx, S_yy, S_xy  over its slab
The host combines partials in float64 via
  HSIC(Ka,Kb) = S_ab - (2/n) ra.rb + (sum Ka)(sum Kb)/n^2
which is algebraically identical to sum(center(Ka)*center(Kb)).

Device details:
  - Gram matmuls run in fp8-e4m3 DoubleRow mode (2 contraction rows per
    partition -> 2x PE throughput, 0.5 cycles per output column).
  - Four extra fp8 contraction rows (hi, hi, mid, lo) carry -(|x_j|^2)/2
    as a cascaded-residual decomposition, folded in as one more DR
    matmul per 512-column half, so exp(G/s^2 + bias_i) with
    bias_i = -|x_i|^2/(2 s^2) yields the full RBF kernel in a single
    ScalarE activation whose accum_out produces row sums for free.
  - PSUM chunks are [128, 2048] (4 banks, double buffered = all 8).
  - Work is phased Y-c0, X-c0, Y-c1, X-c1 (chunk-outer) with DMA in the
    same order, so the three Hadamard-product sums start as early as
    possible and overlap the Gram/exp pipeline.  They run on VectorE
    (scalar_tensor_tensor with fp32 accum), except the last few sum
    Kx*Kx chunks, which run as ScalarE Square activations to fill ACT's
    tail while VectorE drains (GpSimd rejects TensorScalarPtr on HW).
"""

import numpy as np
import ml_dtypes

BF16 = ml_dtypes.bfloat16
FP8 = ml_dtypes.float8_e4m3

N = 4096          # samples
D = 768           # features
NCORES = 8
ROWS = N // NCORES        # 512 rows per core
MT = ROWS // 128          # 4 m-tiles per core
KC2 = D // 256            # 3 DoubleRow contraction chunks (256 rows each)
ACHUNK = 2048             # ACT/psum chunk width (4 PSUM banks)
NCH = N // ACHUNK         # 2 chunks per m-tile row
MMN = 512                 # matmul moving free dim (one PSUM bank)
NHALF = ACHUNK // MMN     # 4 matmul halves per chunk
NACC = MT * NCH           # row-sum accumulator columns (8)
SACC = MT * 4             # product accumulator columns (4 parts per slot)

_cache = {}
LAST_RESULTS = None   # BassKernelResults of the most recent run (for test harness)


def _build(inv_sigma_sq: float, reps: int = 1, stages: str = "all"):
    import concourse.bacc as bacc
    import concourse.mybir as mybir
    import concourse.tile as tile

    fp32 = mybir.dt.float32
    bf16 = mybir.dt.bfloat16
    fp8 = mybir.dt.float8e4
    DR = mybir.MatmulPerfMode.DoubleRow
    Exp = mybir.ActivationFunctionType.Exp
    mult = mybir.AluOpType.mult

    nc = bacc.Bacc(None)

    xt = nc.dram_tensor("xt", [KC2, 128, 2, N], fp8, kind="ExternalInput")
    yt = nc.dram_tensor("yt", [KC2, 128, 2, N], fp8, kind="ExternalInput")
    # stationary slabs, partition-major so each loads in ONE dma
    xbt = nc.dram_tensor("xbt", [128, KC2, 2, ROWS], fp8, kind="ExternalInput")
    ybt = nc.dram_tensor("ybt", [128, KC2, 2, ROWS], fp8, kind="ExternalInput")
    # aug rows for x (cols 0:N), y (N:2N), plus all-ones stationary (2N:2N+128)
    augc = nc.dram_tensor("augc", [2, 2, 2 * N + 128], fp8,
                          kind="ExternalInput")
    biasc = nc.dram_tensor("biasc", [128, 2 * MT], fp32, kind="ExternalInput")

    outs_o = nc.dram_tensor("outs", [128, 2 * NACC + 3 * SACC], fp32,
                          kind="ExternalOutput")

    with tile.TileContext(nc) as tc:
        with (
            tc.tile_pool(name="res", bufs=1) as res,
            tc.tile_pool(name="kmat", bufs=1) as kpool,
            tc.tile_pool(name="scr", bufs=1) as spool,
            tc.tile_pool(name="psum", bufs=2, space="PSUM") as pp,
        ):
            # ---- persistent tiles ----
            t_sb = {}      # moving operands [128, 2, N] per chunk
            bt_sb = {}     # stationary slabs [128, KC2, 2, ROWS]
            for mat in ("x", "y"):
                for k in range(KC2):
                    t_sb[mat, k] = res.tile([128, 2, N], fp8,
                                            tag=f"{mat}t{k}", name=f"{mat}t{k}")
                bt_sb[mat] = res.tile([128, KC2, 2, ROWS], fp8,
                                      tag=f"{mat}bt", name=f"{mat}bt")
            augc_sb = res.tile([2, 2, 2 * N + 128], fp8, tag="augc",
                               name="augc_sb")
            biasc_sb = res.tile([128, 2 * MT], fp32, tag="biasc",
                                name="biasc_sb")
            aug_sb = {"x": augc_sb[:, :, 0:N], "y": augc_sb[:, :, N:2 * N]}
            augst_ap = augc_sb[:, :, 2 * N:2 * N + 128]
            bias_sb = {"x": biasc_sb[:, 0:MT], "y": biasc_sb[:, MT:2 * MT]}

            acc = res.tile([128, 2 * NACC + 3 * SACC], fp32, tag="acc",
                           name="acc")
            racc = {"x": acc[:, 0:NACC], "y": acc[:, NACC:2 * NACC]}
            sxx_acc = acc[:, 2 * NACC:2 * NACC + SACC]
            syy_acc = acc[:, 2 * NACC + SACC:2 * NACC + 2 * SACC]
            sxy_acc = acc[:, 2 * NACC + 2 * SACC:2 * NACC + 3 * SACC]

            def load():
                # Constants + stationaries first, then moving columns in
                # compute-phase order: Y-lo, X-lo, Y-hi, X-hi.
                nc.scalar.dma_start(biasc_sb[:], biasc[:])
                nc.scalar.dma_start(augc_sb[:], augc[:])
                nc.sync.dma_start(bt_sb["y"][:], ybt[:])
                sl0 = slice(0, ACHUNK)
                for k in range(KC2):
                    nc.sync.dma_start(t_sb["y", k][:, :, sl0], yt[k][:, :, sl0])
                nc.sync.dma_start(bt_sb["x"][:], xbt[:])
                for k in range(KC2):
                    nc.sync.dma_start(t_sb["x", k][:, :, sl0], xt[k][:, :, sl0])
                sl1 = slice(ACHUNK, N)
                for mat, tdram in (("y", yt), ("x", xt)):
                    for k in range(KC2):
                        nc.sync.dma_start(t_sb[mat, k][:, :, sl1],
                                          tdram[k][:, :, sl1])

            def gram_exp_ap(mat, m, out_ap, nch):
                """RBF kernel chunk: rows [m*128,(m+1)*128) x cols chunk nch,
                written to out_ap ([128, ACHUNK])."""
                g = pp.tile([128, ACHUNK], fp32, tag="g", name="g")
                for k in range(KC2):
                    stat = bt_sb[mat][:, k, :, m * 128:(m + 1) * 128]
                    for h in range(NHALF):
                        base = nch * ACHUNK + h * MMN
                        nc.tensor.matmul(
                            g[:, h * MMN:(h + 1) * MMN],
                            stat,
                            t_sb[mat, k][:, :, base:base + MMN],
                            start=(k == 0),
                            stop=False,
                            perf_mode=DR,
                        )
                for h in range(NHALF):
                    base = nch * ACHUNK + h * MMN
                    nc.tensor.matmul(
                        g[:, h * MMN:(h + 1) * MMN],
                        augst_ap,
                        aug_sb[mat][:, :, base:base + MMN],
                        start=False,
                        stop=True,
                        perf_mode=DR,
                    )
                col = m * NCH + nch
                nc.scalar.activation(
                    out_ap,
                    g[:],
                    Exp,
                    bias=bias_sb[mat][:, m:m + 1],
                    scale=inv_sigma_sq,
                    accum_out=racc[mat][:, col:col + 1],
                )

            def body():
                if stages in ("all", "dma"):
                    load()
                if stages == "dma":
                    return

                ky = {
                    m: kpool.tile([128, N], bf16, tag=f"ky{m}", name=f"ky{m}")
                    for m in range(MT)
                }
                kx = {
                    m: kpool.tile([128, N], bf16, tag=f"kx{m}", name=f"kx{m}")
                    for m in range(MT)
                }

                def part_ranges(m, nch):
                    """Symmetric product sub-ranges of chunk `nch` for slot m.

                    With each core's moving columns pre-rotated by -c*ROWS,
                    slot m's tile column j' maps to global column tile
                    (4c + m + j'//128 - m) ... i.e. offset d = j'//128 - m.
                    Products cover d = 0..16: d=0 (own diagonal block) and
                    d=16 (computed by both mirror owners) at weight 1,
                    d=1..15 (mirror owner skips) at weight 2.
                    Returns (start, width, acc_col) triples within `nch`.
                    """
                    out = []
                    if nch == 0:
                        out.append((m * 128, 128, m * 4 + 0))          # d=0
                        out.append(((m + 1) * 128,
                                    ACHUNK - (m + 1) * 128, m * 4 + 1))
                    else:
                        w2b = (m + 16) * 128 - ACHUNK                  # d<16 tail
                        if w2b > 0:
                            out.append((ACHUNK, w2b, m * 4 + 2))
                        out.append(((m + 16) * 128, 128, m * 4 + 3))   # d=16
                    return out

                def prods(m, nch, pairs):
                    """pairs: list of (in0_tile, in1_tile, acc_tile)."""
                    for st, w, col in part_ranges(m, nch):
                        for in0, in1, acc in pairs:
                            dscr = spool.tile([128, ACHUNK], bf16,
                                              tag="dscr", name="dscr",
                                              bufs=3)
                            nc.vector.scalar_tensor_tensor(
                                out=dscr[:, 0:w],
                                in0=in0[:, st:st + w], scalar=1.0,
                                in1=in1[:, st:st + w], op0=mult, op1=mult,
                                accum_out=acc[:, col:col + 1],
                            )

                def do_y(m, nch):
                    sl = slice(nch * ACHUNK, (nch + 1) * ACHUNK)
                    gram_exp_ap("y", m, ky[m][:, sl], nch)
                    prods(m, nch, [(ky[m], ky[m], syy_acc)])

                def do_x(m, nch):
                    sl = slice(nch * ACHUNK, (nch + 1) * ACHUNK)
                    gram_exp_ap("x", m, kx[m][:, sl], nch)
                    prods(m, nch, [(kx[m], kx[m], sxx_acc),
                                   (kx[m], ky[m], sxy_acc)])

                # Chunk phase 0: Y leads by two m-tiles (its data lands
                # first); X m-tiles interleave as xt-lo arrives.
                order0 = [("y", 0), ("y", 1), ("x", 0), ("y", 2), ("x", 1),
                          ("y", 3), ("x", 2), ("x", 3)]
                order1 = [("y", 0), ("x", 0), ("y", 1), ("x", 1), ("y", 2),
                          ("x", 2), ("y", 3), ("x", 3)]
                for nch, order in ((0, order0), (1, order1)):
                    for mat, m in order:
                        if mat == "y":
                            do_y(m, nch)
                        else:
                            do_x(m, nch)

            if stages == "compute":
                load()
            if reps == 1:
                body()
            elif reps < 0:          # unrolled: -reps copies, no hw loop
                for _ in range(-reps):
                    body()
            else:
                with tc.For_i(0, reps, 1):
                    body()

            if stages != "dma":
                nc.sync.dma_start(outs_o[:], acc[:])

    if not nc.is_finalized():
        nc.finalize()
    return nc


def _prep_matrix(A, inv_sigma_sq):
    """Host-side: fp8 cast, transpose+DoubleRow layout, fp8 aug rows, bias."""
    A8 = A.astype(FP8)
    Af = A8.astype(np.float64)
    d = (Af ** 2).sum(axis=1)                             # [N] row norms^2
    AT = np.ascontiguousarray(A8.T)                       # [D, N] fp8

    # -(d_j)/2 as 4 cascaded fp8 rows (a0 + a0 + a2 + a3), |a0| <= 240
    a0 = (-0.25 * d).astype(FP8)
    rem = -0.5 * d - 2.0 * a0.astype(np.float64)
    a2 = rem.astype(FP8)
    rem2 = rem - a2.astype(np.float64)
    a3 = rem2.astype(FP8)
    aug = np.empty((2, 2, N), dtype=FP8)
    aug[0, 0] = a0
    aug[0, 1] = a0
    aug[1, 0] = a2
    aug[1, 1] = a3

    # bias uses the SAME fp8-cascade value as the aug rows so the computed
    # exponent (G + ahat_j)/s^2 + ahat_i/s^2 is symmetric in (i, j) — the
    # symmetric product reconstruction double-counts one triangle, so any
    # aug-vs-bias mismatch would break K's symmetry at the ~1e-3 level.
    ahat = (2.0 * a0.astype(np.float64) + a2.astype(np.float64)
            + a3.astype(np.float64))                      # ~= -d/2
    bias = (inv_sigma_sq * ahat).astype(np.float32)       # [N]
    return AT, aug, bias


def _dr_layout(AT_slice):
    """[768, W] fp8 -> [KC2, 128, 2, W] DoubleRow layout (row = i*128+p)."""
    W = AT_slice.shape[1]
    return np.ascontiguousarray(
        AT_slice.reshape(KC2, 2, 128, W).transpose(0, 2, 1, 3))


def _make_in_maps(X, Y, inv_sigma_sq):
    XT, xaug, xbias = _prep_matrix(X, inv_sigma_sq)
    YT, yaug, ybias = _prep_matrix(Y, inv_sigma_sq)
    xt_r = _dr_layout(XT)
    yt_r = _dr_layout(YT)

    augc = np.ones((2, 2, 2 * N + 128), dtype=FP8)
    augc[:, :, 0:N] = xaug
    augc[:, :, N:2 * N] = yaug

    in_maps = []
    for c in range(NCORES):
        sl = slice(c * ROWS, (c + 1) * ROWS)
        biasc = np.empty((128, 2 * MT), dtype=np.float32)
        biasc[:, 0:MT] = xbias[sl].reshape(MT, 128).T
        biasc[:, MT:2 * MT] = ybias[sl].reshape(MT, 128).T
        # Rotate this core's moving columns left by c*ROWS so the
        # symmetric product ranges [m*128, (m+17)*128) are the same AP
        # on every core (kernel column j' = global (j' + c*ROWS) % N).
        sh = -c * ROWS
        augc_c = augc.copy()
        augc_c[:, :, 0:N] = np.roll(xaug, sh, axis=-1)
        augc_c[:, :, N:2 * N] = np.roll(yaug, sh, axis=-1)
        in_maps.append({
            "xt": np.ascontiguousarray(np.roll(xt_r, sh, axis=-1)),
            "yt": np.ascontiguousarray(np.roll(yt_r, sh, axis=-1)),
            "xbt": np.ascontiguousarray(
                _dr_layout(XT[:, sl]).transpose(1, 0, 2, 3)),
            "ybt": np.ascontiguousarray(
                _dr_layout(YT[:, sl]).transpose(1, 0, 2, 3)),
            "augc": augc_c,
            "biasc": np.ascontiguousarray(biasc),
        })
    return in_maps


def _combine(out):
    rx = np.empty(N, dtype=np.float64)
    ry = np.empty(N, dtype=np.float64)
    s_xx = s_yy = s_xy = 0.0
    for c in range(NCORES):
        o = out[c]["outs"].astype(np.float64)
        r = {
            "rx": o[:, 0:NACC],
            "ry": o[:, NACC:2 * NACC],
            "sxx": o[:, 2 * NACC:2 * NACC + SACC],
            "syy": o[:, 2 * NACC + SACC:2 * NACC + 2 * SACC],
            "sxy": o[:, 2 * NACC + 2 * SACC:],
        }
        rxc = r["rx"].reshape(128, MT, NCH).sum(axis=2)
        ryc = r["ry"].reshape(128, MT, NCH).sum(axis=2)
        rx[c * ROWS:(c + 1) * ROWS] = rxc.T.reshape(ROWS)
        ry[c * ROWS:(c + 1) * ROWS] = ryc.T.reshape(ROWS)
        # product accs: 4 parts per slot with symmetry weights
        # (own-diagonal block 1, offsets 1..15 doubled, offset 16 once).
        # Slot 0's part 2 (w2b) is empty and never written on device —
        # mask out whatever stale SBUF contents it holds.
        w = np.tile(np.array([1.0, 2.0, 2.0, 1.0]), MT)
        w[2] = 0.0
        s_xx += (r["sxx"].astype(np.float64).sum(axis=0) * w).sum()
        s_yy += (r["syy"].astype(np.float64).sum(axis=0) * w).sum()
        s_xy += (r["sxy"].astype(np.float64).sum(axis=0) * w).sum()

    tx = rx.sum()
    ty = ry.sum()
    n = float(N)
    hsic_xy = s_xy - 2.0 / n * np.dot(rx, ry) + tx * ty / (n * n)
    hsic_xx = s_xx - 2.0 / n * np.dot(rx, rx) + tx * tx / (n * n)
    hsic_yy = s_yy - 2.0 / n * np.dot(ry, ry) + ty * ty / (n * n)
    return np.float32(hsic_xy / np.sqrt(hsic_xx * hsic_yy))


def kernel(X, Y, sigma, _reps=1):
    import os
    from concourse.bass_utils import run_bass_kernel_spmd

    X = np.asarray(X, dtype=np.float32)
    Y = np.asarray(Y, dtype=np.float32)
    sig = float(np.asarray(sigma))
    inv_sigma_sq = 1.0 / (sig * sig)

    stages = os.environ.get("KERNEL_STAGES", "all")
    key = (inv_sigma_sq, _reps, stages)
    if key not in _cache:
        _cache[key] = _build(inv_sigma_sq, reps=_reps, stages=stages)
    nc = _cache[key]

    in_maps = _make_in_maps(X, Y, inv_sigma_sq)
    res = run_bass_kernel_spmd(nc, in_maps, list(range(NCORES)))
    global LAST_RESULTS
    LAST_RESULTS = res
    return _combine(res.results)
